# revision 31
# baseline (speedup 1.0000x reference)
"""Chamfer-distance 1-NN kernel for TRN2, sharded over 8 NeuronCores.

Problem (B=1, D=256, S=96, N=S*S=9216):
  Xf = l2norm_cols(X); Yf = l2norm_cols(Y)   [D, N]
  sim = Xf.T @ Yf; idx = argmax(sim, -1) (first occurrence)
  Y_sel = Yf[:, idx]; loss = mean((Xf - Y_sel)**2)
Returns (loss, Y_sel[1,D,N], Xf[1,D,N]).

Sharding: queries split 8 ways (1152/core); Y replicated.

Per core:
  - column norms via ones-vector matmuls over ACT-squared tiles (fp32)
  - Xf/Yf scaled fp32; Yf chunks PE-transposed into DRAM yft[N, D]
  - sim via the float32r hi/lo split: hi = round-to-f32r(v) (DVE copy),
    lo = v - hi; sim = xh@yh + xh@yl + xl@yh accumulated in PSUM.
    f32r matmuls run at 1 cyc/row (4x fp32) and are self-loading (no
    Ldweights). Measured max err 7e-8 vs this input's 4.07e-6 minimum
    top-2 gap -> argmax identical to fp32.
  - N processed in two 4608-halves (the f32r splits of a half fit SBUF);
    per half: DVE Max + MaxIndex with the half-local max as needle; a
    final arithmetic combine picks the global first-occurrence argmax.
  - Y_sel rows gathered from yft by indirect DMA (u32 row indices)
  - loss via mean|Xf-Ysel|^2 = (2N - 2*sum(max sim))/(D*N) on unit
    columns; host sums the per-query maxima in fp64.

DMA discipline: each DMA costs ~1.3us serial on its sequencer plus a
shared HWDGE slot, so transfers are consolidated (row-contiguous loads,
grouped transpose blocks) and spread across SP / ACT / Pool queues.
"""
import numpy as np

D = 256
N = 9216
NCORES = 8
NQ = N // NCORES          # 1152
RT = NQ // 128            # 9 row tiles
KC = D // 128             # 2 contraction chunks
HALF = N // 2             # 4608
MTH = HALF // 512         # 9 m tiles per half
CH = 1536                 # y panel width (3 per kc per half, = 3 m-tiles)
NB = CH // 128            # 12 transpose blocks per panel
NP = HALF // CH           # 3 panels per half per kc

_CACHE = {}


def _build():
    if "nc" in _CACHE:
        return _CACHE["nc"]
    import concourse.bass as bass
    import concourse.bacc as bacc
    import concourse.mybir as mybir
    from concourse import tile

    f32 = mybir.dt.float32
    f32r = mybir.dt.float32r
    u32 = mybir.dt.uint32
    Op = mybir.AluOpType
    Act = mybir.ActivationFunctionType

    nc = bacc.Bacc("TRN2", target_bir_lowering=False, debug=False,
                   num_devices=NCORES)

    xc_d = nc.dram_tensor("xc", [D, NQ], f32, kind="ExternalInput")
    y_d = nc.dram_tensor("y", [D, N], f32, kind="ExternalInput")
    ones_d = nc.dram_tensor("ones", [128, 1], f32, kind="ExternalInput")
    ident_d = nc.dram_tensor("ident", [128, 128], f32, kind="ExternalInput")

    xf_d = nc.dram_tensor("xf", [D, NQ], f32, kind="ExternalOutput")
    yselt_d = nc.dram_tensor("yselt", [NQ, D], f32, kind="ExternalOutput")
    mv_d = nc.dram_tensor("mv", [128, RT], f32, kind="ExternalOutput")
    nnidx_d = nc.dram_tensor("nnidx", [128, RT], u32, kind="ExternalOutput")

    yft_d = nc.dram_tensor("yft", [N, D], f32, kind="Internal")
    rny_d = nc.dram_tensor("rny", [N], f32, kind="Internal")
    rnx_d = nc.dram_tensor("rnx", [NQ], f32, kind="Internal")

    with tile.TileContext(nc) as tc:
        with (
            tc.tile_pool(name="sb", bufs=1) as pool,
            tc.tile_pool(name="sqp", bufs=2) as sqp,
            tc.tile_pool(name="evp", bufs=2) as evp,
            tc.tile_pool(name="simp", bufs=3) as simp,
            tc.tile_pool(name="bcp", bufs=2) as bcp,
            tc.tile_pool(name="ychp", bufs=2) as ychp,
            tc.tile_pool(name="psS", bufs=4, space="PSUM") as psS,
            tc.tile_pool(name="psT", bufs=2, space="PSUM") as psT,
            tc.tile_pool(name="psN", bufs=2, space="PSUM") as psN,
        ):
            ones_t = pool.tile([128, 1], f32, tag="ones")
            nc.sync.dma_start(ones_t[:], ones_d.ap())
            ident_t = pool.tile([128, 128], f32, tag="ident")
            nc.sync.dma_start(ident_t[:], ident_d.ap())

            # ================= X phase =================
            xt = []
            for kc in range(KC):
                t = ychp.tile([128, CH], f32, tag="ych", bufs=4)
                nc.sync.dma_start(
                    t[:, :NQ], bass.AP(xc_d, kc * 128 * NQ, [[NQ, 128], [1, NQ]]))
                xt.append(t[:, :NQ])

            xev = evp.tile([1, 1536], f32, tag="n2ev", bufs=2)
            for mt3 in range(3):
                sl = slice(mt3 * 384, (mt3 + 1) * 384)
                xs = []
                for kc in range(KC):
                    s = sqp.tile([128, 512], f32, tag="ysq")
                    nc.vector.tensor_tensor(
                        s[:, :384], xt[kc][:, sl], xt[kc][:, sl], op=Op.mult)
                    xs.append(s)
                psx = psN.tile([1, 512], f32, tag="psn2")
                for kc in range(KC):
                    nc.tensor.matmul(psx[:, :384], ones_t[:],
                                     xs[kc][:, :384],
                                     start=(kc == 0), stop=(kc == KC - 1))
                nc.scalar.copy(xev[:, mt3 * 384:(mt3 + 1) * 384], psx[:, :384])
            nc.scalar.activation(xev[:, :NQ], xev[:, :NQ], Act.Sqrt)
            nc.vector.reciprocal(xev[:, :NQ], xev[:, :NQ])
            nc.scalar.dma_start(
                bass.AP(rnx_d, 0, [[NQ, 1], [1, NQ]]), xev[:, :NQ])

            bcx = bcp.tile([128, CH], f32, tag="bc", bufs=2)
            nc.sync.dma_start(
                bcx[:, :NQ], bass.AP(rnx_d, 0, [[0, 128], [1, NQ]]))
            xhr, xlr = [], []
            for kc in range(KC):
                nc.vector.tensor_tensor(
                    xt[kc][:], xt[kc][:], bcx[:, :NQ], op=Op.mult)
                nc.scalar.dma_start(
                    bass.AP(xf_d, kc * 128 * NQ, [[NQ, 128], [1, NQ]]),
                    xt[kc][:])
                h = pool.tile([128, NQ], f32r, tag=f"xhr{kc}")
                nc.vector.tensor_copy(h[:], xt[kc][:])
                l = pool.tile([128, NQ], f32r, tag=f"xlr{kc}")
                nc.vector.tensor_tensor(l[:], xt[kc][:], h[:], op=Op.subtract)
                xhr.append(h)
                xlr.append(l)

            # ============ all-panel column norms -> rny_d ============
            for pi in range(N // CH):      # 6 panels globally
                coff = pi * CH
                ycs = []
                for kc in range(KC):
                    yc = ychp.tile([128, CH], f32, tag="ych", bufs=4)
                    dma = nc.sync.dma_start if kc == 0 else nc.scalar.dma_start
                    dma(yc[:], bass.AP(y_d, kc * 128 * N + coff,
                                       [[N, 128], [1, CH]]))
                    ycs.append(yc)
                n2ev = evp.tile([1, 1536], f32, tag="n2ev", bufs=2)
                for q6 in range(CH // 384):
                    qsl = slice(q6 * 384, (q6 + 1) * 384)
                    xs = []
                    for kc in range(KC):
                        sq = sqp.tile([128, 512], f32, tag="ysq")
                        nc.vector.tensor_tensor(
                            sq[:, :384], ycs[kc][:, qsl], ycs[kc][:, qsl],
                            op=Op.mult)
                        xs.append(sq)
                    psy = psN.tile([1, 512], f32, tag="psn2")
                    for kc in range(KC):
                        nc.tensor.matmul(
                            psy[:, :384], ones_t[:], xs[kc][:, :384],
                            start=(kc == 0), stop=(kc == KC - 1))
                    nc.scalar.copy(
                        n2ev[:, q6 * 384:(q6 + 1) * 384], psy[:, :384])
                nc.scalar.activation(n2ev[:, :CH], n2ev[:, :CH], Act.Sqrt)
                nc.vector.reciprocal(n2ev[:, :CH], n2ev[:, :CH])
                nc.scalar.dma_start(
                    bass.AP(rny_d, coff, [[CH, 1], [1, CH]]),
                    n2ev[:, :CH])

            # ============ two half-passes: scale/split/transpose + sim ====
            mvh = []
            idxf = []
            for h in range(1):
                t = pool.tile([128, RT], f32, tag=f"mvh{h}")
                u = pool.tile([128, RT], f32, tag=f"idxf{h}")
                mvh.append(t)
                idxf.append(u)
            mv_t = pool.tile([128, RT], f32, tag="mv")
            nnidx_t = pool.tile([128, RT], u32, tag="nnidx")

            for half in range(2):
                hbase = half * HALF
                yhr = [[], []]
                ylr = [[], []]
                for ci in range(NP):           # 3 panels per half
                    coff = hbase + ci * CH
                    bc = bcp.tile([128, CH], f32, tag="bc", bufs=2)
                    nc.sync.dma_start(
                        bc[:], bass.AP(rny_d, coff, [[0, 128], [1, CH]]))
                    for kc in range(KC):
                        yc = ychp.tile([128, CH], f32, tag="ych", bufs=4)
                        dma = nc.sync.dma_start if kc == 0 else nc.scalar.dma_start
                        dma(yc[:], bass.AP(y_d, kc * 128 * N + coff,
                                           [[N, 128], [1, CH]]))
                        nc.vector.tensor_tensor(yc[:], yc[:], bc[:], op=Op.mult)
                        hh = pool.tile([128, CH], f32r, tag=f"yhr{kc}", bufs=3)
                        nc.vector.tensor_copy(hh[:], yc[:])
                        ll = pool.tile([128, CH], f32r, tag=f"ylr{kc}", bufs=3)
                        nc.vector.tensor_tensor(ll[:], yc[:], hh[:],
                                                op=Op.subtract)
                        yhr[kc].append(hh)
                        ylr[kc].append(ll)
                        for gb in range(0, NB, 6):
                            ev = evp.tile([128, 768], f32, tag="tev")
                            for b in range(6):
                                pst = psT.tile([128, 128], f32, tag="pst")
                                c0 = (gb + b) * 128
                                nc.tensor.transpose(
                                    pst[:], yc[:, c0:c0 + 128], ident_t[:])
                                nc.scalar.copy(
                                    ev[:, b * 128:(b + 1) * 128], pst[:])
                            nc.gpsimd.dma_start(
                                bass.AP(yft_d,
                                        (coff + gb * 128) * D + kc * 128,
                                        [[D, 128], [128 * D, 6], [1, 128]]),
                                ev[:])

                # sim for this half
                for rt in range(RT):
                    rsl = slice(rt * 128, (rt + 1) * 128)
                    sim = simp.tile([128, HALF], f32, tag="sim")
                    for mt in range(MTH):
                        msl = slice(mt * 512, (mt + 1) * 512)
                        ps = psS.tile([128, 512], f32, tag="psS")
                        pn, lsl = mt // 3, slice((mt % 3) * 512,
                                                 (mt % 3 + 1) * 512)
                        terms = []
                        for kc in range(KC):
                            terms += [(xhr[kc], yhr[kc][pn]),
                                      (xhr[kc], ylr[kc][pn]),
                                      (xlr[kc], yhr[kc][pn])]
                        for i, (a, b) in enumerate(terms):
                            nc.tensor.matmul(
                                ps[:], a[:, rsl], b[:, lsl],
                                start=(i == 0), stop=(i == len(terms) - 1))
                        nc.scalar.copy(sim[:, msl], ps[:])
                    top8 = evp.tile([128, 8], f32, tag="top8")
                    nc.vector.max(top8[:], sim[:])
                    idx8 = evp.tile([128, 8], u32, tag="idx8")
                    nc.vector.max_index(idx8[:], top8[:], sim[:])
                    if half == 0:
                        nc.vector.tensor_copy(
                            mvh[0][:, rt:rt + 1], top8[:, 0:1])
                        nc.vector.tensor_copy(
                            idxf[0][:, rt:rt + 1], idx8[:, 0:1])
                    else:
                        # combine with pass-1 result and gather now
                        ibf = evp.tile([128, 1], f32, tag="ibf")
                        nc.vector.tensor_copy(ibf[:], idx8[:, 0:1])
                        nc.vector.tensor_scalar_add(ibf[:], ibf[:], float(HALF))
                        cmp1 = evp.tile([128, 1], mybir.dt.uint8, tag="cmp1")
                        nc.vector.tensor_tensor(
                            cmp1[:], mvh[0][:, rt:rt + 1], top8[:, 0:1],
                            op=Op.is_ge)
                        self_idx = evp.tile([128, 1], f32, tag="selidx")
                        nc.vector.select(
                            self_idx[:], cmp1[:], idxf[0][:, rt:rt + 1], ibf[:])
                        nc.vector.tensor_tensor(
                            mv_t[:, rt:rt + 1], mvh[0][:, rt:rt + 1],
                            top8[:, 0:1], op=Op.max)
                        idxu = evp.tile([128, 1], u32, tag="idxu")
                        nc.vector.tensor_copy(idxu[:], self_idx[:])
                        nc.vector.tensor_copy(
                            nnidx_t[:, rt:rt + 1], idxu[:])
                        gat = evp.tile([128, D], f32, tag="gat")
                        nc.gpsimd.indirect_dma_start(
                            gat[:], None, yft_d.ap(),
                            bass.IndirectOffsetOnAxis(ap=idxu[:], axis=0))
                        nc.scalar.dma_start(
                            bass.AP(yselt_d, rt * 128 * D, [[D, 128], [1, D]]),
                            gat[:])

            nc.sync.dma_start(mv_d.ap(), mv_t[:])
            nc.sync.dma_start(nnidx_d.ap(), nnidx_t[:])

    nc.compile()
    _CACHE["nc"] = nc
    return nc


def kernel(X_features, Y_features, image_x, image_y):
    from concourse.bass_utils import run_bass_kernel_spmd

    X = np.ascontiguousarray(
        np.asarray(X_features, dtype=np.float32).reshape(D, N))
    Y = np.ascontiguousarray(
        np.asarray(Y_features, dtype=np.float32).reshape(D, N))

    nc = _build()
    ones = np.ones((128, 1), dtype=np.float32)
    ident = np.eye(128, dtype=np.float32)
    in_maps = [
        {
            "xc": np.ascontiguousarray(X[:, c * NQ:(c + 1) * NQ]),
            "y": Y,
            "ones": ones,
            "ident": ident,
        }
        for c in range(NCORES)
    ]
    res = run_bass_kernel_spmd(nc, in_maps, list(range(NCORES)))
    rs = res.results

    xf = np.concatenate([r["xf"] for r in rs], axis=1)
    ysel = np.concatenate([r["yselt"].T for r in rs], axis=1)
    ysel = np.ascontiguousarray(ysel)
    S = sum(float(r["mv"].astype(np.float64).sum()) for r in rs)
    loss = np.float32((2.0 * N - 2.0 * S) / (D * N))
    return (loss, ysel[None], xf[None])
